# revision 45
# baseline (speedup 1.0000x reference)
"""Trainium2 Bass kernel: causal self-attention (B=2, T=2048, D=1024, H=16).

NOTE: the reference's window constraint `(key - query) < 16` is vacuous under
causality, so this is FULL causal attention over T=2048 per batch.

Sharding: 8 cores = 2 batches x 4 head-groups (4 heads each). Every core
runs the identical program on (its batch's x^T, its group's W columns):
  - Q^T/K^T [256e, 2048t] and V [2048t, 256e] projections (bf16 inputs,
    fp32 accumulation)
  - blocked causal attention per head: 256-query chunks against 128-key
    blocks; exp on ScalarE without max-subtraction (scores are O(1));
    softmax denominator via an appended ones-column in V (row 64 of the
    PV matmul); diagonal trim: the fully-masked lower half of key block
    2qc+1 is never computed/exp'd/streamed, and only two 128-wide
    triangles get mask muls
  - partial output projection y_g = Ot_g^T @ Wo_g^T  [2048, 1024]
The host sums the 4 per-group partial y's of each batch (no device
collectives) and stacks the 2 batches.

Matmul operands are bf16 (1 PE cycle/row, fp32 PSUM accumulation). fp8
was evaluated and rejected: quantization error is multiplicative through
the bilinear ops (no softmax washout) and lands at 3-5e-2 rel error vs
the 2e-2 gate.

Key scheduling ideas vs the naive ordering:
  - The Q^T/K^T projections are NOT a serial prologue. They are emitted
    chunk-wise (512-token chunks) and interleaved with the attention
    stream, so the first exp lands on the Scalar engine within ~8us
    instead of ~53us; iteration (qc, a) only needs q/k tokens <
    (qc+1)*256. x^T streams in per-chunk behind wk so the PE starts at
    the DMA-preamble floor.
  - All cross-engine consumers (U chunks, z-normalize chains, out-proj
    chunks) lag their producers by ~one iteration so semaphore latency
    hides under other PE work.
  - The iteration order ends (..., (7,1), (1,1), (0,1)): the serial tail
    (last exp -> U -> normalize -> out-proj -> store) runs on the two
    smallest chunks.

`loop_reps > 0` builds a timing variant with the whole body inside a
hardware For_i loop (used to measure per-execution HW time despite
multi-ms dispatch jitter).
"""

import numpy as np

# Problem shapes (hardcoded; kernel.py must be self-contained)
B, T, D = 2, 2048, 1024
H, HD = 16, 64
NCORES = 8
NG = 4                       # head groups
HG = H // NG                 # 4 heads per group
EG = HG * HD                 # 256 embedding cols per group
P = 128
CS = D // P                  # 8 contraction subtiles for Q/K/V projections
QCH = 256                    # query chunk
NQC = T // QCH               # 8 query chunks
NKB = T // P                 # 16 key blocks
TCH = 512                    # token chunk for the q/k projections
NTC = T // TCH               # 4 projection token chunks

_nc_cache = {}


def _emit_body(nc, env, variant="full"):
    """Emit one full forward pass (projections + attention + out-proj)."""
    import concourse.mybir as mybir

    f32 = mybir.dt.float32
    bf16 = mybir.dt.bfloat16
    Exp = mybir.ActivationFunctionType.Exp
    xt, wq, wk, wv, wo, md, y_d = (env[k] for k in
                                   ("xt", "wq", "wk", "wv", "wo", "md", "y_d"))
    bigp, ebufp, workp, zp, pp, sp, up = (env[k] for k in
                                          ("bigp", "ebufp", "workp", "zp",
                                           "pp", "sp", "up"))
    mm = nc.tensor.matmul

    # ---- Q^T / K^T projection chunks: [e_local on partitions, t free],
    # emitted as schedulable units (one 512-token half-slab each) ----
    qt = bigp.tile([P, 2, T], bf16, tag="qt", name="qt")
    kt = bigp.tile([P, 2, T], bf16, tag="kt", name="kt")

    def qk_unit(which, et, tc):
        dst, w_sb = (kt, wk) if which == "k" else (qt, wq)
        ps = pp.tile([P, TCH], f32, tag="proj", name="ps_p")
        for s in range(CS):
            mm(ps, w_sb[:, s, et * P:(et + 1) * P],
               xt[:, s, tc * TCH:(tc + 1) * TCH],
               start=(s == 0), stop=(s == CS - 1))
        nc.vector.tensor_copy(
            out=dst[:, et, tc * TCH:(tc + 1) * TCH], in_=ps)

    # ---- V: [t on partitions, head, 64+1] with ones column;
    # emitted chunk-wise inside the attention stream as PE filler ----
    vt = bigp.tile([P, NKB, HG, HD + 1], bf16, tag="vt", name="vt")
    nc.vector.memset(vt[:, :, :, HD:HD + 1], 1.0)

    def vchunk(kb):
        ps = pp.tile([P, TCH], f32, tag="proj", name="ps_v")
        for s in range(CS):
            mm(ps[:, :EG], xt[:, s, kb * P:(kb + 1) * P], wv[:, s, :],
               start=(s == 0), stop=(s == CS - 1))
        nc.vector.tensor_copy(
            out=vt[:, kb, :, 0:HD],
            in_=ps[:, :EG].rearrange("p (j d) -> p j d", d=HD))

    # ---- blocked causal attention ----
    # Per head, query chunk qc uses key blocks 0..2qc+1. Key blocks come
    # in pairs sharing one PSUM bank; only the final (diagonal) pair is
    # masked after exp.
    ot = bigp.tile([P, 2, T], bf16, tag="ot", name="ot")
    if variant in ("noattn", "nou"):
        nc.vector.memset(ot, 0.0)
    estate = {}

    def attn_s_pair(it, kbp, e):
        """4 S matmuls (both heads x 2 key blocks) + one exp; the
        diagonal pair skips its fully-masked regions (block 2qc+1 is
        entirely invalid for the chunk's first 128 queries) and masks
        only the two 128-wide triangles."""
        qc, a = it
        qs = qc * QCH
        diag = kbp == qc
        # one 2-bank psum tile: bank0 = head 2a, bank1 = head 2a+1
        s4 = sp.tile([P, 2, 2 * QCH], f32, tag="s4", name="s4")
        for half in (0, 1):
            q0 = P if (diag and half == 1) else 0
            for hh in (0, 1):
                po = 64 * hh
                qsl = qt[po:po + 64, a, qs + q0:qs + QCH]
                kb = 2 * kbp + half
                mm(s4[:, hh, half * QCH + q0:(half + 1) * QCH],
                   kt[po:po + 64, a, kb * P:(kb + 1) * P], qsl,
                   start=(half == 0), stop=(half == 1),
                   skip_group_check=True)
        if not diag:
            nc.scalar.activation(out=e[:, kbp, :, :, :], in_=s4,
                                 func=Exp, scale=0.125)
            return
        # diagonal: exp only the written regions (block 2qc full width,
        # block 2qc+1 upper half), then mask the two triangles
        nc.scalar.activation(out=e[:, qc, :, 0, :], in_=s4[:, :, 0:QCH],
                             func=Exp, scale=0.125)
        nc.scalar.activation(out=e[:, qc, :, 1, P:QCH],
                             in_=s4[:, :, QCH + P:2 * QCH],
                             func=Exp, scale=0.125)
        nc.vector.tensor_mul(e[:, qc, :, 0, 0:P],
                             e[:, qc, :, 0, 0:P], md[:, 0, :, :])
        nc.vector.tensor_mul(e[:, qc, :, 1, P:QCH],
                             e[:, qc, :, 1, P:QCH], md[:, 1, :, :])

    # PV accumulation: one [65, 256] PSUM tile per (iteration, head);
    # row 64 is the softmax denominator from the V ones-column.
    ustate = {}
    udone = {}     # (it, hh) -> key blocks already emitted (own-drain)

    def attn_u_chunk(it, hh, k0, k1):
        """PV accumulation chunk [k0,k1) for one head of (qc, a)."""
        qc, a = it
        e = estate[it]
        h = 2 * a + hh
        nkb = 2 * qc + 2
        if (it, hh) not in ustate:
            ustate[(it, hh)] = up.tile([HD + 1, QCH], f32, tag="u", name="u")
        u = ustate[(it, hh)]
        for kb in range(k0, k1):
            if kb == nkb - 1:
                # diagonal odd block: only the chunk's upper 128 queries
                # attend to it (its e lower half was never written)
                mm(u[:, P:QCH], vt[:, kb, h, :],
                   e[:, kb // 2, hh, 1, P:QCH],
                   start=False, stop=True)
            else:
                mm(u, vt[:, kb, h, :], e[:, kb // 2, hh, kb % 2, :],
                   start=(kb == 0), stop=False)

    def attn_norm(it):
        """Softmax-normalize both heads of (qc, a): one shared
        z-reciprocal/broadcast chain, then two per-head muls into ot."""
        qc, a = it
        qs = qc * QCH
        u0 = ustate.pop((it, 0))
        u1 = ustate.pop((it, 1))
        # custom-DVE ops must read SBUF (PSUM reads corrupt when the op
        # is instantiated repeatedly) — stage the denominator rows first
        zs = zp.tile([1, 2, QCH], f32, tag="zs", name="zs")
        nc.vector.tensor_copy(out=zs[:, 0, :], in_=u0[HD:HD + 1, :])
        nc.vector.tensor_copy(out=zs[:, 1, :], in_=u1[HD:HD + 1, :])
        zr = zp.tile([1, 2, QCH], f32, tag="zr", name="zr")
        nc.vector.reciprocal_approx_fast(zr, zs)
        zb = zp.tile([HD, 2, QCH], f32, tag="zb", name="zb")
        nc.gpsimd.partition_broadcast(zb, zr)
        for hh, u in ((0, u0), (1, u1)):
            nc.vector.tensor_mul(ot[64 * hh:64 * hh + 64, a, qs:qs + QCH],
                                 u[0:HD, :], zb[:, hh, :])

    def outproj_half(tc16, eh):
        """Half of the partial output projection for one 128-token chunk:
        y_g[tc, eh] = Ot_g[tc]^T @ Wo_g[:, eh]^T (one 512-wide e half)."""
        ps = pp.tile([P, 512], f32, tag="proj", name="ps_o")
        for s in range(2):
            mm(ps, ot[:, s, tc16 * P:(tc16 + 1) * P],
               wo[:, s, eh * 512:(eh + 1) * 512],
               start=(s == 0), stop=(s == 1))
        ysb = workp.tile([P, 512], bf16, tag="ysb", name="ysb")
        if eh == 0:
            nc.scalar.copy(ysb, ps)
        else:
            nc.vector.tensor_copy(out=ysb, in_=ps)
        nc.sync.dma_start(
            y_d[:][tc16 * P:(tc16 + 1) * P, eh * 512:(eh + 1) * 512], ysb)

    def outproj_chunk(tc16):
        outproj_half(tc16, 0)
        outproj_half(tc16, 1)

    from collections import deque
    # projection units in dataflow-priority order; iteration (qc, a) can
    # only be emitted after units covering tokens < (qc+1)*256 (i.e.
    # chunks <= tc_need(qc)) for BOTH q and k
    projq = deque((w, et, tc) for tc in range(NTC)
                  for w in ("k", "q") for et in range(2))
    proj_tc_done = -1          # last tc with all 4 units emitted

    def emit_proj():
        nonlocal proj_tc_done
        w, et, tc = projq.popleft()
        qk_unit(w, et, tc)
        if not projq or projq[0][2] != tc:
            proj_tc_done = tc

    vq = deque(range(NKB))

    def tc_need(qc):
        return ((qc + 1) * QCH - 1) // TCH

    if variant == "noattn":
        while projq:
            emit_proj()
        for kb in range(NKB):
            vchunk(kb)
        for t in range(16):
            outproj_chunk(t)
        return

    # Merged emission: iteration i's S pairs interleaved with PE-filler
    # units whose dependencies are already settled: remaining projection
    # chunks, V-projection chunks, U chunks + normalize of iteration i-1,
    # ready out-proj halves. Units carry PE-time estimates (ns) and are
    # drip-fed between S pairs so each pair's exp (~1.1us on ACT) drains
    # while the PE chews filler.
    # Interleaved ascending order, but (0, 1) moved to the very end: the
    # kernel finishes on a 1-pair iteration, so the serial tail (last exp
    # -> U -> normalize -> out-proj -> store) is as short as possible,
    # and (7, 1)'s big U drain + the last out-proj chunks serve as PE
    # filler under the final exps. Since (qc, 0) always normalizes before
    # (qc, 1), out-proj chunks for qc unlock right after (qc, 1)'s
    # normalize.
    iters = ([(0, 0), (1, 0)]
             + [(qc, a) for qc in range(2, NQC) for a in (0, 1)]
             + [(1, 1), (0, 1)])
    UMM = 270          # ns per U matmul (N=256 bf16 at observed clock)
    QKU = 1750         # ns per q/k projection unit
    EXP_NS = 1150      # ACT exp drain time per S pair

    def units_for(i):
        """Filler units for iteration i: previous iteration's U chunks +
        normalize, projection/V chunks, out-proj halves — budget-fed
        between the S pairs so the PE never idles while ACT drains exps."""
        qc = iters[i][0]
        us = []        # (est_ns, fn)
        if i >= 1 and variant != "nou":
            prev = iters[i - 1]
            nkb = 2 * prev[0] + 2
            # V chunks consumed by U(prev) must be emitted first
            while vq and vq[0] < nkb:
                us.append((2150, lambda kb=vq.popleft(): vchunk(kb)))
            for hh in (0, 1):
                for k0 in range(udone.get((prev, hh), 0), nkb, 4):
                    k1 = min(nkb, k0 + 4)
                    us.append((UMM * (k1 - k0),
                               lambda p=prev, hh=hh, k0=k0, k1=k1:
                               attn_u_chunk(p, hh, k0, k1)))
            us.append((600, lambda p=prev: attn_norm(p)))
        # out-proj chunks lag their unlocking normalize by one iteration:
        # emitting them right after the norm would chain PE -> DVE ->
        # Pool -> DVE -> PE with ~1us of semaphore latency stalling the
        # PE at every chunk
        ready = []
        if i >= 2 and iters[i - 2][1] == 1:
            ready = [iters[i - 2][0]]
        if i == len(iters) - 1 and variant != "nou":
            # final iteration: drain its own (tiny) U + normalize right
            # after the previous iteration's, so the last two z-chains
            # overlap the remaining out-proj matmuls instead of
            # serializing after them
            lit = iters[i]
            lnkb = 2 * lit[0] + 2
            for hh in (0, 1):
                us.append((UMM * lnkb,
                           lambda hh=hh:
                           attn_u_chunk(lit, hh,
                                        udone.get((lit, hh), 0), lnkb)))
            us.append((600, lambda: attn_norm(lit)))
            if iters[i - 1][1] == 1:
                ready.append(iters[i - 1][0])
            ready.append(lit[0])
        if variant != "nou":
            for qq in ready:
                for t in (2 * qq, 2 * qq + 1):
                    for eh in (0, 1):
                        us.append((1100,
                                   lambda t=t, eh=eh: outproj_half(t, eh)))
        # drip remaining projection chunks ahead of need (hard backstop
        # below keeps correctness)
        lookahead = tc_need(min(qc + 1, NQC - 1)) + 1
        n_drip = sum(1 for u in projq if u[2] < lookahead)
        for _ in range(min(n_drip, 6)):
            us.append((QKU, emit_proj))
        if not projq:
            for _ in range(2):
                if vq:
                    us.append((2150, lambda kb=vq.popleft(): vchunk(kb)))
        return us

    for i, it in enumerate(iters):
        qc, a = it
        # hard backstop: q/k chunks this iteration reads must exist
        while proj_tc_done < tc_need(qc):
            emit_proj()
        e = ebufp.tile([P, NQC, 2, 2, QCH], bf16, tag="ebuf", name="ebuf")
        estate[it] = e
        units = units_for(i)
        npairs = qc + 1
        total = sum(c for c, _ in units)
        # per-pair filler budget: enough to drain one exp, and spread so
        # all units are consumed by the end of the iteration
        per_pair = max(EXP_NS, -(-total // npairs))
        done = 0
        spent = 0
        # diagonal pair FIRST among S pairs: its mask-mul waits on its exp
        # and would head-of-line-block the vector queue for the chains
        # queued after it
        pair_order = [qc] + list(range(qc))
        for pi, kbp in enumerate(pair_order):
            attn_s_pair(it, kbp, e)
            while done < len(units) and spent < (pi + 1) * per_pair:
                c, fn = units[done]
                fn()
                spent += c
                done += 1
        while done < len(units):
            units[done][1]()
            done += 1
        if i >= 2:
            estate.pop(iters[i - 2], None)
    if variant == "nou":
        for t in range(2 * NQC):
            outproj_chunk(t)
    estate.clear()


def _build_nc(loop_reps=0, variant="full"):
    from contextlib import nullcontext

    import concourse.bacc as bacc
    import concourse.tile as tile
    import concourse.mybir as mybir

    f32 = mybir.dt.float32
    bf16 = mybir.dt.bfloat16

    nc = bacc.Bacc("TRN2", target_bir_lowering=False, debug=False)

    # weights arrive pre-swizzled host-side into partition-major layout so
    # each DMA moves contiguous 4KB lines per partition (fast descriptors)
    xt_d = nc.dram_tensor("xt", [D, T], bf16, kind="ExternalInput")
    wq_d = nc.dram_tensor("wqt", [P, CS * EG], bf16, kind="ExternalInput")
    wk_d = nc.dram_tensor("wkt", [P, CS * EG], bf16, kind="ExternalInput")
    wv_d = nc.dram_tensor("wvt", [P, CS * EG], bf16, kind="ExternalInput")
    wo_d = nc.dram_tensor("wot", [P, 2 * D], bf16, kind="ExternalInput")
    md_d = nc.dram_tensor("maskd", [P, 2, 2, P], bf16, kind="ExternalInput")
    y_d = nc.dram_tensor("y", [T, D], bf16, kind="ExternalOutput")

    with tile.TileContext(nc) as tc:
        with (
            tc.tile_pool(name="const", bufs=1) as constp,
            tc.tile_pool(name="big", bufs=1) as bigp,
            tc.tile_pool(name="ebuf", bufs=3) as ebufp,
            tc.tile_pool(name="work", bufs=3) as workp,
            tc.tile_pool(name="zs", bufs=6) as zp,
            tc.tile_pool(name="pp", bufs=2, space="PSUM") as pp,
            tc.tile_pool(name="sp", bufs=2, space="PSUM") as sp,
            tc.tile_pool(name="up", bufs=2, space="PSUM") as up,
        ):
            # ---- load W slices, x^T (c on partitions), diagonal masks.
            # Issue order: first matmuls need wk/wq + xt token-chunk 0;
            # the rest of xt streams in chunk by chunk behind them. ----
            wq = constp.tile([P, CS, EG], bf16, name="wq")
            wk = constp.tile([P, CS, EG], bf16, name="wk")
            wv = constp.tile([P, CS, EG], bf16, name="wv")
            wo = constp.tile([P, 2, D], bf16, name="wo")
            md = constp.tile([P, 2, 2, P], bf16, name="md")
            wk_r = wk_d[:].rearrange("p (s e) -> p s e", s=CS)
            nc.sync.dma_start(wk[:, 0:2, :], wk_r[:, 0:2, :])
            nc.sync.dma_start(wk[:, 2:CS, :], wk_r[:, 2:CS, :])

            xt = bigp.tile([P, CS, T], bf16, name="xt")
            xt_r = xt_d[:].rearrange("(s p) t -> p s t", p=P)
            # token-chunk 0 lands slice-by-slice right behind wk so the
            # first projection matmuls start ~2us in; wq/md/wv follow in
            # first-use order
            for s in range(CS):
                nc.sync.dma_start(xt[:, s, 0:TCH], xt_r[:, s, 0:TCH])
            nc.sync.dma_start(wq, wq_d[:].rearrange("p (s e) -> p s e", s=CS))
            nc.sync.dma_start(md, md_d[:])
            nc.sync.dma_start(wv, wv_d[:].rearrange("p (s e) -> p s e", s=CS))
            for tcc in range(1, NTC):
                nc.sync.dma_start(xt[:, :, tcc * TCH:(tcc + 1) * TCH],
                                  xt_r[:, :, tcc * TCH:(tcc + 1) * TCH])
            nc.sync.dma_start(wo, wo_d[:].rearrange("p (s e) -> p s e", s=2))

            env = dict(xt=xt, wq=wq, wk=wk, wv=wv, wo=wo, md=md, y_d=y_d,
                       bigp=bigp, ebufp=ebufp, workp=workp, zp=zp,
                       pp=pp, sp=sp, up=up)
            if loop_reps:
                with tc.For_i(0, loop_reps, 1):
                    _emit_body(nc, env, variant)
            else:
                _emit_body(nc, env, variant)

    nc.compile()
    return nc


def get_nc(loop_reps=0, variant="full"):
    key = (loop_reps, variant)
    if key not in _nc_cache:
        _nc_cache[key] = _build_nc(loop_reps, variant)
    return _nc_cache[key]


def _diag_masks():
    """[P, block, hh, 128] triangles: block 0 = key block 2qc against the
    chunk's first 128 queries; block 1 = key block 2qc+1 against the
    chunk's upper 128 queries. (Everything else on the diagonal pair is
    either fully valid or never computed.) Same mask for both heads."""
    jk = np.arange(P)[:, None]
    i = np.arange(P)[None, :]
    md = np.empty((P, 2, 2, P), np.float32)
    md[:, 0, :, :] = (jk <= i)[:, None]
    md[:, 1, :, :] = (jk <= i)[:, None]
    return md


def _pack_pm(w, nsub):
    """[nsub*128, F] -> partition-major [128, nsub*F] (row p = concat over
    sub-blocks of row sub*128+p) so the device DMA is contiguous per
    partition."""
    n, f = w.shape
    assert n == nsub * P
    return np.ascontiguousarray(
        w.reshape(nsub, P, f).transpose(1, 0, 2).reshape(P, nsub * f))


def prep_inputs(x, Wq, Wk, Wv, Wo):
    import ml_dtypes
    bf16 = ml_dtypes.bfloat16
    x = np.asarray(x, np.float32)
    md = _diag_masks()
    wqt = np.asarray(Wq, np.float32).T    # [c, e]
    wkt = np.asarray(Wk, np.float32).T
    wvt = np.asarray(Wv, np.float32).T
    wot = np.asarray(Wo, np.float32).T    # [c, e]

    in_maps = []
    for c in range(NCORES):
        b, g = divmod(c, NG)
        e0 = g * EG
        in_maps.append({
            "xt": np.ascontiguousarray(x[b].T).astype(bf16),
            "wqt": _pack_pm(wqt[:, e0:e0 + EG], CS).astype(bf16),
            "wkt": _pack_pm(wkt[:, e0:e0 + EG], CS).astype(bf16),
            "wvt": _pack_pm(wvt[:, e0:e0 + EG], CS).astype(bf16),
            "wot": _pack_pm(wot[e0:e0 + EG, :], 2).astype(bf16),
            "maskd": md.astype(bf16),
        })
    return in_maps


def gather_output(results):
    ys = [np.asarray(r["y"], np.float32) for r in results]
    out = np.empty((B, T, D), np.float32)
    for b in range(B):
        out[b] = sum(ys[b * NG:(b + 1) * NG])
    return out


last_run = {}


def kernel(x, Wq, Wk, Wv, Wo, _trace=False):
    from concourse.bass_utils import run_bass_kernel_spmd

    nc = get_nc()
    in_maps = prep_inputs(x, Wq, Wk, Wv, Wo)
    res = run_bass_kernel_spmd(
        nc, in_maps, core_ids=list(range(NCORES)), trace=_trace)
    last_run["results"] = res
    return gather_output(res.results)


# revision 46
# speedup vs baseline: 1.1883x; 1.1883x over previous
"""Trainium2 Bass kernel: causal self-attention (B=2, T=2048, D=1024, H=16).

NOTE: the reference's window constraint `(key - query) < 16` is vacuous under
causality, so this is FULL causal attention over T=2048 per batch.

Sharding: 8 cores = 2 batches x 4 head-groups (4 heads each). Every core
runs the identical program on (its batch's x^T, its group's W columns):
  - Q^T/K^T [256e, 2048t] and V [2048t, 256e] projections (bf16 inputs,
    fp32 accumulation)
  - blocked causal attention per head: 256-query chunks against 128-key
    blocks; exp on ScalarE without max-subtraction (scores are O(1));
    softmax denominator via an appended ones-column in V (row 64 of the
    PV matmul); diagonal trim: the fully-masked lower half of key block
    2qc+1 is never computed/exp'd/streamed, and only two 128-wide
    triangles get mask muls
  - partial output projection y_g = Ot_g^T @ Wo_g^T  [2048, 1024]
The host sums the 4 per-group partial y's of each batch (no device
collectives) and stacks the 2 batches.

Matmul operands are bf16 (1 PE cycle/row, fp32 PSUM accumulation). fp8
was evaluated and rejected: quantization error is multiplicative through
the bilinear ops (no softmax washout) and lands at 3-5e-2 rel error vs
the 2e-2 gate.

Key scheduling ideas vs the naive ordering:
  - The Q^T/K^T projections are NOT a serial prologue. They are emitted
    chunk-wise (512-token chunks) and interleaved with the attention
    stream, so the first exp lands on the Scalar engine within ~8us
    instead of ~53us; iteration (qc, a) only needs q/k tokens <
    (qc+1)*256. x^T streams in per-chunk behind wk so the PE starts at
    the DMA-preamble floor.
  - All cross-engine consumers (U chunks, z-normalize chains, out-proj
    chunks) lag their producers by ~one iteration so semaphore latency
    hides under other PE work.
  - The iteration order ends (..., (7,1), (1,1), (0,1)): the serial tail
    (last exp -> U -> normalize -> out-proj -> store) runs on the two
    smallest chunks.

`loop_reps > 0` builds a timing variant with the whole body inside a
hardware For_i loop (used to measure per-execution HW time despite
multi-ms dispatch jitter).
"""

import numpy as np

# Problem shapes (hardcoded; kernel.py must be self-contained)
B, T, D = 2, 2048, 1024
H, HD = 16, 64
NCORES = 8
NG = 4                       # head groups
HG = H // NG                 # 4 heads per group
EG = HG * HD                 # 256 embedding cols per group
P = 128
CS = D // P                  # 8 contraction subtiles for Q/K/V projections
QCH = 256                    # query chunk
NQC = T // QCH               # 8 query chunks
NKB = T // P                 # 16 key blocks
TCH = 512                    # token chunk for the q/k projections
NTC = T // TCH               # 4 projection token chunks

_nc_cache = {}


def _emit_body(nc, env, variant="full"):
    """Emit one full forward pass (projections + attention + out-proj)."""
    import concourse.mybir as mybir

    f32 = mybir.dt.float32
    bf16 = mybir.dt.bfloat16
    Exp = mybir.ActivationFunctionType.Exp
    xt, wq, wk, wv, wo, md, y_d = (env[k] for k in
                                   ("xt", "wq", "wk", "wv", "wo", "md", "y_d"))
    bigp, ebufp, workp, zp, pp, sp, up = (env[k] for k in
                                          ("bigp", "ebufp", "workp", "zp",
                                           "pp", "sp", "up"))
    mm = nc.tensor.matmul

    # ---- Q^T / K^T projection chunks: [e_local on partitions, t free],
    # emitted as schedulable units (one 512-token half-slab each) ----
    qt = bigp.tile([P, 2, T], bf16, tag="qt", name="qt")
    kt = bigp.tile([P, 2, T], bf16, tag="kt", name="kt")

    def qk_unit(which, et, tc):
        dst, w_sb = (kt, wk) if which == "k" else (qt, wq)
        ps = pp.tile([P, TCH], f32, tag="proj", name="ps_p")
        for s in range(CS):
            mm(ps, w_sb[:, s, et * P:(et + 1) * P],
               xt[:, s, tc * TCH:(tc + 1) * TCH],
               start=(s == 0), stop=(s == CS - 1))
        nc.vector.tensor_copy(
            out=dst[:, et, tc * TCH:(tc + 1) * TCH], in_=ps)

    # ---- V: [t on partitions, head, 64+1] with ones column;
    # emitted chunk-wise inside the attention stream as PE filler ----
    vt = bigp.tile([P, NKB, HG, HD + 1], bf16, tag="vt", name="vt")
    nc.vector.memset(vt[:, :, :, HD:HD + 1], 1.0)

    def vchunk(kb):
        ps = pp.tile([P, TCH], f32, tag="proj", name="ps_v")
        for s in range(CS):
            mm(ps[:, :EG], xt[:, s, kb * P:(kb + 1) * P], wv[:, s, :],
               start=(s == 0), stop=(s == CS - 1))
        nc.vector.tensor_copy(
            out=vt[:, kb, :, 0:HD],
            in_=ps[:, :EG].rearrange("p (j d) -> p j d", d=HD))

    # ---- blocked causal attention ----
    # Per head, query chunk qc uses key blocks 0..2qc+1. Key blocks come
    # in pairs sharing one PSUM bank; only the final (diagonal) pair is
    # masked after exp.
    ot = bigp.tile([P, 2, T], bf16, tag="ot", name="ot")
    if variant in ("noattn", "nou"):
        nc.vector.memset(ot, 0.0)
    estate = {}

    def attn_s_pair(it, kbp, e):
        """4 S matmuls (both heads x 2 key blocks) + one exp; the
        diagonal pair skips its fully-masked regions (block 2qc+1 is
        entirely invalid for the chunk's first 128 queries) and masks
        only the two 128-wide triangles."""
        qc, a = it
        qs = qc * QCH
        diag = kbp == qc
        # one 2-bank psum tile: bank0 = head 2a, bank1 = head 2a+1
        s4 = sp.tile([P, 2, 2 * QCH], f32, tag="s4", name="s4")
        for half in (0, 1):
            q0 = P if (diag and half == 1) else 0
            for hh in (0, 1):
                po = 64 * hh
                qsl = qt[po:po + 64, a, qs + q0:qs + QCH]
                kb = 2 * kbp + half
                mm(s4[:, hh, half * QCH + q0:(half + 1) * QCH],
                   kt[po:po + 64, a, kb * P:(kb + 1) * P], qsl,
                   start=(half == 0), stop=(half == 1),
                   skip_group_check=True)
        if not diag:
            nc.scalar.activation(out=e[:, kbp, :, :, :], in_=s4,
                                 func=Exp, scale=0.125)
            return
        # diagonal: exp only the written regions (block 2qc full width,
        # block 2qc+1 upper half), then mask the two triangles
        nc.scalar.activation(out=e[:, qc, :, 0, :], in_=s4[:, :, 0:QCH],
                             func=Exp, scale=0.125)
        nc.scalar.activation(out=e[:, qc, :, 1, P:QCH],
                             in_=s4[:, :, QCH + P:2 * QCH],
                             func=Exp, scale=0.125)
        nc.vector.tensor_mul(e[:, qc, :, 0, 0:P],
                             e[:, qc, :, 0, 0:P], md[:, 0, :, :])
        nc.vector.tensor_mul(e[:, qc, :, 1, P:QCH],
                             e[:, qc, :, 1, P:QCH], md[:, 1, :, :])

    # PV accumulation: one [65, 256] PSUM tile per (iteration, head);
    # row 64 is the softmax denominator from the V ones-column.
    ustate = {}
    udone = {}     # (it, hh) -> key blocks already emitted (own-drain)

    def attn_u_chunk(it, hh, k0, k1):
        """PV accumulation chunk [k0,k1) for one head of (qc, a)."""
        qc, a = it
        e = estate[it]
        h = 2 * a + hh
        nkb = 2 * qc + 2
        if (it, hh) not in ustate:
            ustate[(it, hh)] = up.tile([HD + 1, QCH], f32, tag="u", name="u")
        u = ustate[(it, hh)]
        for kb in range(k0, k1):
            if kb == nkb - 1:
                # diagonal odd block: only the chunk's upper 128 queries
                # attend to it (its e lower half was never written)
                mm(u[:, P:QCH], vt[:, kb, h, :],
                   e[:, kb // 2, hh, 1, P:QCH],
                   start=False, stop=True)
            else:
                mm(u, vt[:, kb, h, :], e[:, kb // 2, hh, kb % 2, :],
                   start=(kb == 0), stop=False)

    def attn_norm(it):
        """Softmax-normalize both heads of (qc, a): one shared
        z-reciprocal/broadcast chain, then two per-head muls into ot."""
        qc, a = it
        qs = qc * QCH
        u0 = ustate.pop((it, 0))
        u1 = ustate.pop((it, 1))
        # custom-DVE ops must read SBUF (PSUM reads corrupt when the op
        # is instantiated repeatedly) — stage the denominator rows first
        zs = zp.tile([1, 2, QCH], f32, tag="zs", name="zs")
        nc.vector.tensor_copy(out=zs[:, 0, :], in_=u0[HD:HD + 1, :])
        nc.vector.tensor_copy(out=zs[:, 1, :], in_=u1[HD:HD + 1, :])
        zr = zp.tile([1, 2, QCH], f32, tag="zr", name="zr")
        nc.vector.reciprocal_approx_fast(zr, zs)
        zb = zp.tile([HD, 2, QCH], f32, tag="zb", name="zb")
        nc.gpsimd.partition_broadcast(zb, zr)
        for hh, u in ((0, u0), (1, u1)):
            nc.vector.tensor_mul(ot[64 * hh:64 * hh + 64, a, qs:qs + QCH],
                                 u[0:HD, :], zb[:, hh, :])

    def outproj_half(tc16, eh):
        """Half of the partial output projection for one 128-token chunk:
        y_g[tc, eh] = Ot_g[tc]^T @ Wo_g[:, eh]^T (one 512-wide e half)."""
        ps = pp.tile([P, 512], f32, tag="proj", name="ps_o")
        for s in range(2):
            mm(ps, ot[:, s, tc16 * P:(tc16 + 1) * P],
               wo[:, s, eh * 512:(eh + 1) * 512],
               start=(s == 0), stop=(s == 1))
        ysb = workp.tile([P, 512], bf16, tag="ysb", name="ysb")
        if eh == 0:
            nc.scalar.copy(ysb, ps)
        else:
            nc.vector.tensor_copy(out=ysb, in_=ps)
        nc.sync.dma_start(
            y_d[:][tc16 * P:(tc16 + 1) * P, eh * 512:(eh + 1) * 512], ysb)

    def outproj_chunk(tc16):
        outproj_half(tc16, 0)
        outproj_half(tc16, 1)

    from collections import deque
    # projection units in dataflow-priority order; iteration (qc, a) can
    # only be emitted after units covering tokens < (qc+1)*256 (i.e.
    # chunks <= tc_need(qc)) for BOTH q and k
    projq = deque((w, et, tc) for tc in range(NTC)
                  for w in ("k", "q") for et in range(2))
    proj_tc_done = -1          # last tc with all 4 units emitted

    def emit_proj():
        nonlocal proj_tc_done
        w, et, tc = projq.popleft()
        qk_unit(w, et, tc)
        if not projq or projq[0][2] != tc:
            proj_tc_done = tc

    vq = deque(range(NKB))

    def tc_need(qc):
        return ((qc + 1) * QCH - 1) // TCH

    if variant == "noattn":
        while projq:
            emit_proj()
        for kb in range(NKB):
            vchunk(kb)
        for t in range(16):
            outproj_chunk(t)
        return

    # Merged emission: iteration i's S pairs interleaved with PE-filler
    # units whose dependencies are already settled: remaining projection
    # chunks, V-projection chunks, U chunks + normalize of iteration i-1,
    # ready out-proj halves. Units carry PE-time estimates (ns) and are
    # drip-fed between S pairs so each pair's exp (~1.1us on ACT) drains
    # while the PE chews filler.
    # Interleaved ascending order, but (0, 1) moved to the very end: the
    # kernel finishes on a 1-pair iteration, so the serial tail (last exp
    # -> U -> normalize -> out-proj -> store) is as short as possible,
    # and (7, 1)'s big U drain + the last out-proj chunks serve as PE
    # filler under the final exps. Since (qc, 0) always normalizes before
    # (qc, 1), out-proj chunks for qc unlock right after (qc, 1)'s
    # normalize.
    iters = ([(0, 0), (1, 0)]
             + [(qc, a) for qc in range(2, NQC) for a in (0, 1)]
             + [(1, 1), (0, 1)])
    UMM = 270          # ns per U matmul (N=256 bf16 at observed clock)
    QKU = 1750         # ns per q/k projection unit
    EXP_NS = 1150      # ACT exp drain time per S pair

    def units_for(i):
        """Filler units for iteration i: previous iteration's U chunks +
        normalize, projection/V chunks, out-proj halves — budget-fed
        between the S pairs so the PE never idles while ACT drains exps."""
        qc = iters[i][0]
        us = []        # (est_ns, fn)
        if i >= 1 and variant != "nou":
            prev = iters[i - 1]
            nkb = 2 * prev[0] + 2
            # V chunks consumed by U(prev) must be emitted first
            while vq and vq[0] < nkb:
                us.append((2150, lambda kb=vq.popleft(): vchunk(kb)))
            for hh in (0, 1):
                for k0 in range(udone.get((prev, hh), 0), nkb, 4):
                    k1 = min(nkb, k0 + 4)
                    us.append((UMM * (k1 - k0),
                               lambda p=prev, hh=hh, k0=k0, k1=k1:
                               attn_u_chunk(p, hh, k0, k1)))
            us.append((600, lambda p=prev: attn_norm(p)))
        # out-proj chunks lag their unlocking normalize by one iteration:
        # emitting them right after the norm would chain PE -> DVE ->
        # Pool -> DVE -> PE with ~1us of semaphore latency stalling the
        # PE at every chunk
        ready = []
        if i >= 2 and iters[i - 2][1] == 1:
            ready = [iters[i - 2][0]]
        if i == len(iters) - 1 and variant != "nou":
            # final iteration: drain its own (tiny) U + normalize right
            # after the previous iteration's, so the last two z-chains
            # overlap the remaining out-proj matmuls instead of
            # serializing after them
            lit = iters[i]
            lnkb = 2 * lit[0] + 2
            for hh in (0, 1):
                us.append((UMM * lnkb,
                           lambda hh=hh:
                           attn_u_chunk(lit, hh,
                                        udone.get((lit, hh), 0), lnkb)))
            us.append((600, lambda: attn_norm(lit)))
            if iters[i - 1][1] == 1:
                ready.append(iters[i - 1][0])
            ready.append(lit[0])
        if variant != "nou":
            for qq in ready:
                for t in (2 * qq, 2 * qq + 1):
                    for eh in (0, 1):
                        us.append((1100,
                                   lambda t=t, eh=eh: outproj_half(t, eh)))
        # drip remaining projection chunks ahead of need (hard backstop
        # below keeps correctness)
        lookahead = tc_need(min(qc + 1, NQC - 1)) + 1
        n_drip = sum(1 for u in projq if u[2] < lookahead)
        for _ in range(min(n_drip, 6)):
            us.append((QKU, emit_proj))
        if not projq:
            for _ in range(2):
                if vq:
                    us.append((2150, lambda kb=vq.popleft(): vchunk(kb)))
        return us

    for i, it in enumerate(iters):
        qc, a = it
        # hard backstop: q/k chunks this iteration reads must exist
        while proj_tc_done < tc_need(qc):
            emit_proj()
        e = ebufp.tile([P, NQC, 2, 2, QCH], bf16, tag="ebuf", name="ebuf")
        estate[it] = e
        units = units_for(i)
        npairs = qc + 1
        total = sum(c for c, _ in units)
        # per-pair filler budget: enough to drain one exp, and spread so
        # all units are consumed by the end of the iteration
        per_pair = max(EXP_NS, -(-total // npairs))
        done = 0
        spent = 0
        # diagonal pair FIRST among S pairs: its mask-mul waits on its exp
        # and would head-of-line-block the vector queue for the chains
        # queued after it
        pair_order = [qc] + list(range(qc))
        for pi, kbp in enumerate(pair_order):
            attn_s_pair(it, kbp, e)
            while done < len(units) and spent < (pi + 1) * per_pair:
                c, fn = units[done]
                fn()
                spent += c
                done += 1
        while done < len(units):
            units[done][1]()
            done += 1
        if i >= 2:
            estate.pop(iters[i - 2], None)
    if variant == "nou":
        for t in range(2 * NQC):
            outproj_chunk(t)
    estate.clear()


def _build_nc(loop_reps=0, variant="full"):
    from contextlib import nullcontext

    import concourse.bacc as bacc
    import concourse.tile as tile
    import concourse.mybir as mybir

    f32 = mybir.dt.float32
    bf16 = mybir.dt.bfloat16

    nc = bacc.Bacc("TRN2", target_bir_lowering=False, debug=False)

    # weights arrive pre-swizzled host-side into partition-major layout so
    # each DMA moves contiguous 4KB lines per partition (fast descriptors)
    xt_d = nc.dram_tensor("xt", [D, T], bf16, kind="ExternalInput")
    wq_d = nc.dram_tensor("wqt", [P, CS * EG], bf16, kind="ExternalInput")
    wk_d = nc.dram_tensor("wkt", [P, CS * EG], bf16, kind="ExternalInput")
    wv_d = nc.dram_tensor("wvt", [P, CS * EG], bf16, kind="ExternalInput")
    wo_d = nc.dram_tensor("wot", [P, 2 * D], bf16, kind="ExternalInput")
    md_d = nc.dram_tensor("maskd", [P, 2, 2, P], bf16, kind="ExternalInput")
    y_d = nc.dram_tensor("y", [T, D], bf16, kind="ExternalOutput")

    with tile.TileContext(nc) as tc:
        with (
            tc.tile_pool(name="const", bufs=1) as constp,
            tc.tile_pool(name="big", bufs=1) as bigp,
            tc.tile_pool(name="ebuf", bufs=3) as ebufp,
            tc.tile_pool(name="work", bufs=6) as workp,
            tc.tile_pool(name="zs", bufs=6) as zp,
            tc.tile_pool(name="pp", bufs=2, space="PSUM") as pp,
            tc.tile_pool(name="sp", bufs=2, space="PSUM") as sp,
            tc.tile_pool(name="up", bufs=2, space="PSUM") as up,
        ):
            # ---- load W slices, x^T (c on partitions), diagonal masks.
            # Issue order: first matmuls need wk/wq + xt token-chunk 0;
            # the rest of xt streams in chunk by chunk behind them. ----
            wq = constp.tile([P, CS, EG], bf16, name="wq")
            wk = constp.tile([P, CS, EG], bf16, name="wk")
            wv = constp.tile([P, CS, EG], bf16, name="wv")
            wo = constp.tile([P, 2, D], bf16, name="wo")
            md = constp.tile([P, 2, 2, P], bf16, name="md")
            wk_r = wk_d[:].rearrange("p (s e) -> p s e", s=CS)
            nc.sync.dma_start(wk[:, 0:2, :], wk_r[:, 0:2, :])
            nc.sync.dma_start(wk[:, 2:CS, :], wk_r[:, 2:CS, :])

            xt = bigp.tile([P, CS, T], bf16, name="xt")
            xt_r = xt_d[:].rearrange("(s p) t -> p s t", p=P)
            # token-chunk 0 lands slice-by-slice right behind wk so the
            # first projection matmuls start ~2us in; wq/md/wv follow in
            # first-use order
            for s in range(CS):
                nc.sync.dma_start(xt[:, s, 0:TCH], xt_r[:, s, 0:TCH])
            nc.sync.dma_start(wq, wq_d[:].rearrange("p (s e) -> p s e", s=CS))
            nc.sync.dma_start(md, md_d[:])
            nc.sync.dma_start(wv, wv_d[:].rearrange("p (s e) -> p s e", s=CS))
            for tcc in range(1, NTC):
                nc.sync.dma_start(xt[:, :, tcc * TCH:(tcc + 1) * TCH],
                                  xt_r[:, :, tcc * TCH:(tcc + 1) * TCH])
            nc.sync.dma_start(wo, wo_d[:].rearrange("p (s e) -> p s e", s=2))

            env = dict(xt=xt, wq=wq, wk=wk, wv=wv, wo=wo, md=md, y_d=y_d,
                       bigp=bigp, ebufp=ebufp, workp=workp, zp=zp,
                       pp=pp, sp=sp, up=up)
            if loop_reps:
                with tc.For_i(0, loop_reps, 1):
                    _emit_body(nc, env, variant)
            else:
                _emit_body(nc, env, variant)

    nc.compile()
    return nc


def get_nc(loop_reps=0, variant="full"):
    key = (loop_reps, variant)
    if key not in _nc_cache:
        _nc_cache[key] = _build_nc(loop_reps, variant)
    return _nc_cache[key]


def _diag_masks():
    """[P, block, hh, 128] triangles: block 0 = key block 2qc against the
    chunk's first 128 queries; block 1 = key block 2qc+1 against the
    chunk's upper 128 queries. (Everything else on the diagonal pair is
    either fully valid or never computed.) Same mask for both heads."""
    jk = np.arange(P)[:, None]
    i = np.arange(P)[None, :]
    md = np.empty((P, 2, 2, P), np.float32)
    md[:, 0, :, :] = (jk <= i)[:, None]
    md[:, 1, :, :] = (jk <= i)[:, None]
    return md


def _pack_pm(w, nsub):
    """[nsub*128, F] -> partition-major [128, nsub*F] (row p = concat over
    sub-blocks of row sub*128+p) so the device DMA is contiguous per
    partition."""
    n, f = w.shape
    assert n == nsub * P
    return np.ascontiguousarray(
        w.reshape(nsub, P, f).transpose(1, 0, 2).reshape(P, nsub * f))


def prep_inputs(x, Wq, Wk, Wv, Wo):
    import ml_dtypes
    bf16 = ml_dtypes.bfloat16
    x = np.asarray(x, np.float32)
    md = _diag_masks()
    wqt = np.asarray(Wq, np.float32).T    # [c, e]
    wkt = np.asarray(Wk, np.float32).T
    wvt = np.asarray(Wv, np.float32).T
    wot = np.asarray(Wo, np.float32).T    # [c, e]

    in_maps = []
    for c in range(NCORES):
        b, g = divmod(c, NG)
        e0 = g * EG
        in_maps.append({
            "xt": np.ascontiguousarray(x[b].T).astype(bf16),
            "wqt": _pack_pm(wqt[:, e0:e0 + EG], CS).astype(bf16),
            "wkt": _pack_pm(wkt[:, e0:e0 + EG], CS).astype(bf16),
            "wvt": _pack_pm(wvt[:, e0:e0 + EG], CS).astype(bf16),
            "wot": _pack_pm(wot[e0:e0 + EG, :], 2).astype(bf16),
            "maskd": md.astype(bf16),
        })
    return in_maps


def gather_output(results):
    ys = [np.asarray(r["y"], np.float32) for r in results]
    out = np.empty((B, T, D), np.float32)
    for b in range(B):
        out[b] = sum(ys[b * NG:(b + 1) * NG])
    return out


last_run = {}


def kernel(x, Wq, Wk, Wv, Wo, _trace=False):
    from concourse.bass_utils import run_bass_kernel_spmd

    nc = get_nc()
    in_maps = prep_inputs(x, Wq, Wk, Wv, Wo)
    res = run_bass_kernel_spmd(
        nc, in_maps, core_ids=list(range(NCORES)), trace=_trace)
    last_run["results"] = res
    return gather_output(res.results)


# revision 47
# speedup vs baseline: 1.2013x; 1.0109x over previous
"""Trainium2 Bass kernel: causal self-attention (B=2, T=2048, D=1024, H=16).

NOTE: the reference's window constraint `(key - query) < 16` is vacuous under
causality, so this is FULL causal attention over T=2048 per batch.

Sharding: 8 cores = 2 batches x 4 head-groups (4 heads each). Every core
runs the identical program on (its batch's x^T, its group's W columns):
  - Q^T/K^T [256e, 2048t] and V [2048t, 256e] projections (bf16 inputs,
    fp32 accumulation)
  - blocked causal attention per head: 256-query chunks against 128-key
    blocks; exp on ScalarE without max-subtraction (scores are O(1));
    softmax denominator via an appended ones-column in V (row 64 of the
    PV matmul); diagonal trim: the fully-masked lower half of key block
    2qc+1 is never computed/exp'd/streamed, and only two 128-wide
    triangles get mask muls
  - partial output projection y_g = Ot_g^T @ Wo_g^T  [2048, 1024]
The host sums the 4 per-group partial y's of each batch (no device
collectives) and stacks the 2 batches.

Matmul operands are bf16 (1 PE cycle/row, fp32 PSUM accumulation). fp8
was evaluated and rejected: quantization error is multiplicative through
the bilinear ops (no softmax washout) and lands at 3-5e-2 rel error vs
the 2e-2 gate.

Key scheduling ideas vs the naive ordering:
  - The Q^T/K^T projections are NOT a serial prologue. They are emitted
    chunk-wise (512-token chunks) and interleaved with the attention
    stream, so the first exp lands on the Scalar engine within ~8us
    instead of ~53us; iteration (qc, a) only needs q/k tokens <
    (qc+1)*256. x^T streams in per-chunk behind wk so the PE starts at
    the DMA-preamble floor.
  - All cross-engine consumers (U chunks, z-normalize chains, out-proj
    chunks) lag their producers by ~one iteration so semaphore latency
    hides under other PE work.
  - The iteration order ends (..., (7,1), (1,1), (0,1)): the serial tail
    (last exp -> U -> normalize -> out-proj -> store) runs on the two
    smallest chunks.

`loop_reps > 0` builds a timing variant with the whole body inside a
hardware For_i loop (used to measure per-execution HW time despite
multi-ms dispatch jitter).
"""

import numpy as np

# Problem shapes (hardcoded; kernel.py must be self-contained)
B, T, D = 2, 2048, 1024
H, HD = 16, 64
NCORES = 8
NG = 4                       # head groups
HG = H // NG                 # 4 heads per group
EG = HG * HD                 # 256 embedding cols per group
P = 128
CS = D // P                  # 8 contraction subtiles for Q/K/V projections
QCH = 256                    # query chunk
NQC = T // QCH               # 8 query chunks
NKB = T // P                 # 16 key blocks
TCH = 512                    # token chunk for the q/k projections
NTC = T // TCH               # 4 projection token chunks

_nc_cache = {}


def _emit_body(nc, env, variant="full"):
    """Emit one full forward pass (projections + attention + out-proj)."""
    import concourse.mybir as mybir

    f32 = mybir.dt.float32
    bf16 = mybir.dt.bfloat16
    Exp = mybir.ActivationFunctionType.Exp
    xt, wq, wk, wv, wo, md, y_d = (env[k] for k in
                                   ("xt", "wq", "wk", "wv", "wo", "md", "y_d"))
    bigp, ebufp, workp, zp, pp, sp, up = (env[k] for k in
                                          ("bigp", "ebufp", "workp", "zp",
                                           "pp", "sp", "up"))
    mm = nc.tensor.matmul

    # ---- Q^T / K^T projection chunks: [e_local on partitions, t free],
    # emitted as schedulable units (one 512-token half-slab each) ----
    qt = bigp.tile([P, 2, T], bf16, tag="qt", name="qt")
    kt = bigp.tile([P, 2, T], bf16, tag="kt", name="kt")

    def qk_unit(which, et, tc):
        dst, w_sb = (kt, wk) if which == "k" else (qt, wq)
        ps = pp.tile([P, TCH], f32, tag="proj", name="ps_p")
        for s in range(CS):
            mm(ps, w_sb[:, s, et * P:(et + 1) * P],
               xt[:, s, tc * TCH:(tc + 1) * TCH],
               start=(s == 0), stop=(s == CS - 1))
        nc.vector.tensor_copy(
            out=dst[:, et, tc * TCH:(tc + 1) * TCH], in_=ps)

    # ---- V: [t on partitions, head, 64+1] with ones column;
    # emitted chunk-wise inside the attention stream as PE filler ----
    vt = bigp.tile([P, NKB, HG, HD + 1], bf16, tag="vt", name="vt")
    nc.vector.memset(vt[:, :, :, HD:HD + 1], 1.0)

    def vchunk(kb):
        ps = pp.tile([P, TCH], f32, tag="proj", name="ps_v")
        for s in range(CS):
            mm(ps[:, :EG], xt[:, s, kb * P:(kb + 1) * P], wv[:, s, :],
               start=(s == 0), stop=(s == CS - 1))
        nc.vector.tensor_copy(
            out=vt[:, kb, :, 0:HD],
            in_=ps[:, :EG].rearrange("p (j d) -> p j d", d=HD))

    # ---- blocked causal attention ----
    # Per head, query chunk qc uses key blocks 0..2qc+1. Key blocks come
    # in pairs sharing one PSUM bank; only the final (diagonal) pair is
    # masked after exp.
    ot = bigp.tile([P, 2, T], bf16, tag="ot", name="ot")
    if variant in ("noattn", "nou"):
        nc.vector.memset(ot, 0.0)
    estate = {}

    def attn_s_pair(it, kbp, e):
        """4 S matmuls (both heads x 2 key blocks) + one exp; the
        diagonal pair skips its fully-masked regions (block 2qc+1 is
        entirely invalid for the chunk's first 128 queries) and masks
        only the two 128-wide triangles."""
        qc, a = it
        qs = qc * QCH
        diag = kbp == qc
        # one 2-bank psum tile: bank0 = head 2a, bank1 = head 2a+1
        s4 = sp.tile([P, 2, 2 * QCH], f32, tag="s4", name="s4")
        for half in (0, 1):
            q0 = P if (diag and half == 1) else 0
            for hh in (0, 1):
                po = 64 * hh
                qsl = qt[po:po + 64, a, qs + q0:qs + QCH]
                kb = 2 * kbp + half
                mm(s4[:, hh, half * QCH + q0:(half + 1) * QCH],
                   kt[po:po + 64, a, kb * P:(kb + 1) * P], qsl,
                   start=(half == 0), stop=(half == 1),
                   skip_group_check=True)
        if not diag:
            nc.scalar.activation(out=e[:, kbp, :, :, :], in_=s4,
                                 func=Exp, scale=0.125)
            return
        # diagonal: exp only the written regions (block 2qc full width,
        # block 2qc+1 upper half), then mask the two triangles
        nc.scalar.activation(out=e[:, qc, :, 0, :], in_=s4[:, :, 0:QCH],
                             func=Exp, scale=0.125)
        nc.scalar.activation(out=e[:, qc, :, 1, P:QCH],
                             in_=s4[:, :, QCH + P:2 * QCH],
                             func=Exp, scale=0.125)
        nc.vector.tensor_mul(e[:, qc, :, 0, 0:P],
                             e[:, qc, :, 0, 0:P], md[:, 0, :, :])
        nc.vector.tensor_mul(e[:, qc, :, 1, P:QCH],
                             e[:, qc, :, 1, P:QCH], md[:, 1, :, :])

    # PV accumulation: one [65, 256] PSUM tile per (iteration, head);
    # row 64 is the softmax denominator from the V ones-column.
    ustate = {}
    udone = {}     # (it, hh) -> key blocks already emitted (own-drain)

    def attn_u_chunk(it, hh, k0, k1):
        """PV accumulation chunk [k0,k1) for one head of (qc, a)."""
        qc, a = it
        e = estate[it]
        h = 2 * a + hh
        nkb = 2 * qc + 2
        if (it, hh) not in ustate:
            ustate[(it, hh)] = up.tile([HD + 1, QCH], f32, tag="u", name="u")
        u = ustate[(it, hh)]
        for kb in range(k0, k1):
            if kb == nkb - 1:
                # diagonal odd block: only the chunk's upper 128 queries
                # attend to it (its e lower half was never written)
                mm(u[:, P:QCH], vt[:, kb, h, :],
                   e[:, kb // 2, hh, 1, P:QCH],
                   start=False, stop=True)
            else:
                mm(u, vt[:, kb, h, :], e[:, kb // 2, hh, kb % 2, :],
                   start=(kb == 0), stop=False)

    def attn_norm(it):
        """Softmax-normalize both heads of (qc, a): one shared
        z-reciprocal/broadcast chain, then two per-head muls into ot."""
        qc, a = it
        qs = qc * QCH
        u0 = ustate.pop((it, 0))
        u1 = ustate.pop((it, 1))
        # custom-DVE ops must read SBUF (PSUM reads corrupt when the op
        # is instantiated repeatedly) — stage the denominator rows first
        zs = zp.tile([1, 2, QCH], f32, tag="zs", name="zs")
        nc.vector.tensor_copy(out=zs[:, 0, :], in_=u0[HD:HD + 1, :])
        nc.vector.tensor_copy(out=zs[:, 1, :], in_=u1[HD:HD + 1, :])
        zr = zp.tile([1, 2, QCH], f32, tag="zr", name="zr")
        nc.vector.reciprocal_approx_fast(zr, zs)
        zb = zp.tile([HD, 2, QCH], f32, tag="zb", name="zb")
        nc.gpsimd.partition_broadcast(zb, zr)
        for hh, u in ((0, u0), (1, u1)):
            nc.vector.tensor_mul(ot[64 * hh:64 * hh + 64, a, qs:qs + QCH],
                                 u[0:HD, :], zb[:, hh, :])

    def outproj_half(tc16, eh):
        """Half of the partial output projection for one 128-token chunk:
        y_g[tc, eh] = Ot_g[tc]^T @ Wo_g[:, eh]^T (one 512-wide e half)."""
        ps = pp.tile([P, 512], f32, tag="proj", name="ps_o")
        for s in range(2):
            mm(ps, ot[:, s, tc16 * P:(tc16 + 1) * P],
               wo[:, s, eh * 512:(eh + 1) * 512],
               start=(s == 0), stop=(s == 1))
        ysb = workp.tile([P, 512], bf16, tag="ysb", name="ysb")
        if eh == 0:
            nc.scalar.copy(ysb, ps)
        else:
            nc.vector.tensor_copy(out=ysb, in_=ps)
        nc.sync.dma_start(
            y_d[:][tc16 * P:(tc16 + 1) * P, eh * 512:(eh + 1) * 512], ysb)

    def outproj_chunk(tc16):
        outproj_half(tc16, 0)
        outproj_half(tc16, 1)

    from collections import deque
    # projection units in dataflow-priority order; iteration (qc, a) can
    # only be emitted after units covering tokens < (qc+1)*256 (i.e.
    # chunks <= tc_need(qc)) for BOTH q and k
    projq = deque((w, et, tc) for tc in range(NTC)
                  for w in ("k", "q") for et in range(2))
    proj_tc_done = -1          # last tc with all 4 units emitted

    def emit_proj():
        nonlocal proj_tc_done
        w, et, tc = projq.popleft()
        qk_unit(w, et, tc)
        if not projq or projq[0][2] != tc:
            proj_tc_done = tc

    vq = deque(range(NKB))

    def tc_need(qc):
        return ((qc + 1) * QCH - 1) // TCH

    if variant == "noattn":
        while projq:
            emit_proj()
        for kb in range(NKB):
            vchunk(kb)
        for t in range(16):
            outproj_chunk(t)
        return

    # Merged emission: iteration i's S pairs interleaved with PE-filler
    # units whose dependencies are already settled: remaining projection
    # chunks, V-projection chunks, U chunks + normalize of iteration i-1,
    # ready out-proj halves. Units carry PE-time estimates (ns) and are
    # drip-fed between S pairs so each pair's exp (~1.1us on ACT) drains
    # while the PE chews filler.
    # Interleaved ascending order, but (0, 1) moved to the very end: the
    # kernel finishes on a 1-pair iteration, so the serial tail (last exp
    # -> U -> normalize -> out-proj -> store) is as short as possible,
    # and (7, 1)'s big U drain + the last out-proj chunks serve as PE
    # filler under the final exps. Since (qc, 0) always normalizes before
    # (qc, 1), out-proj chunks for qc unlock right after (qc, 1)'s
    # normalize.
    iters = ([(0, 0), (1, 0)]
             + [(qc, a) for qc in range(2, NQC) for a in (0, 1)]
             + [(1, 1), (0, 1)])
    UMM = 270          # ns per U matmul (N=256 bf16 at observed clock)
    QKU = 1750         # ns per q/k projection unit
    EXP_NS = 1350      # ACT exp drain time per S pair (incl. sem latency)

    def units_for(i):
        """Filler units for iteration i: previous iteration's U chunks +
        normalize, projection/V chunks, out-proj halves — budget-fed
        between the S pairs so the PE never idles while ACT drains exps."""
        qc = iters[i][0]
        us = []        # (est_ns, fn)
        if i >= 1 and variant != "nou":
            prev = iters[i - 1]
            nkb = 2 * prev[0] + 2
            # V chunks consumed by U(prev) must be emitted first
            while vq and vq[0] < nkb:
                us.append((2150, lambda kb=vq.popleft(): vchunk(kb)))
            for hh in (0, 1):
                for k0 in range(udone.get((prev, hh), 0), nkb, 4):
                    k1 = min(nkb, k0 + 4)
                    us.append((UMM * (k1 - k0),
                               lambda p=prev, hh=hh, k0=k0, k1=k1:
                               attn_u_chunk(p, hh, k0, k1)))
            us.append((600, lambda p=prev: attn_norm(p)))
        # out-proj chunks lag their unlocking normalize by one iteration:
        # emitting them right after the norm would chain PE -> DVE ->
        # Pool -> DVE -> PE with ~1us of semaphore latency stalling the
        # PE at every chunk
        ready = []
        if i >= 2 and iters[i - 2][1] == 1:
            ready = [iters[i - 2][0]]
        if i == len(iters) - 1 and variant != "nou":
            # final iteration: drain its own (tiny) U + normalize right
            # after the previous iteration's, so the last two z-chains
            # overlap the remaining out-proj matmuls instead of
            # serializing after them
            lit = iters[i]
            lnkb = 2 * lit[0] + 2
            for hh in (0, 1):
                us.append((UMM * lnkb,
                           lambda hh=hh:
                           attn_u_chunk(lit, hh,
                                        udone.get((lit, hh), 0), lnkb)))
            us.append((600, lambda: attn_norm(lit)))
            if iters[i - 1][1] == 1:
                ready.append(iters[i - 1][0])
            ready.append(lit[0])
        if variant != "nou":
            for qq in ready:
                for t in (2 * qq, 2 * qq + 1):
                    for eh in (0, 1):
                        us.append((1100,
                                   lambda t=t, eh=eh: outproj_half(t, eh)))
        # drip remaining projection chunks ahead of need (hard backstop
        # below keeps correctness)
        lookahead = tc_need(min(qc + 1, NQC - 1)) + 1
        n_drip = sum(1 for u in projq if u[2] < lookahead)
        for _ in range(min(n_drip, 6)):
            us.append((QKU, emit_proj))
        if not projq:
            for _ in range(2):
                if vq:
                    us.append((2150, lambda kb=vq.popleft(): vchunk(kb)))
        return us

    for i, it in enumerate(iters):
        qc, a = it
        # hard backstop: q/k chunks this iteration reads must exist
        while proj_tc_done < tc_need(qc):
            emit_proj()
        e = ebufp.tile([P, NQC, 2, 2, QCH], bf16, tag="ebuf", name="ebuf")
        estate[it] = e
        units = units_for(i)
        npairs = qc + 1
        total = sum(c for c, _ in units)
        # per-pair filler budget: enough to drain one exp, and spread so
        # all units are consumed by the end of the iteration
        per_pair = max(EXP_NS, -(-total // npairs))
        done = 0
        spent = 0
        # diagonal pair FIRST among S pairs: its mask-mul waits on its exp
        # and would head-of-line-block the vector queue for the chains
        # queued after it
        pair_order = [qc] + list(range(qc))
        for pi, kbp in enumerate(pair_order):
            attn_s_pair(it, kbp, e)
            while done < len(units) and spent < (pi + 1) * per_pair:
                c, fn = units[done]
                fn()
                spent += c
                done += 1
        while done < len(units):
            units[done][1]()
            done += 1
        if i >= 2:
            estate.pop(iters[i - 2], None)
    if variant == "nou":
        for t in range(2 * NQC):
            outproj_chunk(t)
    estate.clear()


def _build_nc(loop_reps=0, variant="full"):
    from contextlib import nullcontext

    import concourse.bacc as bacc
    import concourse.tile as tile
    import concourse.mybir as mybir

    f32 = mybir.dt.float32
    bf16 = mybir.dt.bfloat16

    nc = bacc.Bacc("TRN2", target_bir_lowering=False, debug=False)

    # weights arrive pre-swizzled host-side into partition-major layout so
    # each DMA moves contiguous 4KB lines per partition (fast descriptors)
    xt_d = nc.dram_tensor("xt", [D, T], bf16, kind="ExternalInput")
    wq_d = nc.dram_tensor("wqt", [P, CS * EG], bf16, kind="ExternalInput")
    wk_d = nc.dram_tensor("wkt", [P, CS * EG], bf16, kind="ExternalInput")
    wv_d = nc.dram_tensor("wvt", [P, CS * EG], bf16, kind="ExternalInput")
    wo_d = nc.dram_tensor("wot", [P, 2 * D], bf16, kind="ExternalInput")
    md_d = nc.dram_tensor("maskd", [P, 2, 2, P], bf16, kind="ExternalInput")
    y_d = nc.dram_tensor("y", [T, D], bf16, kind="ExternalOutput")

    with tile.TileContext(nc) as tc:
        with (
            tc.tile_pool(name="const", bufs=1) as constp,
            tc.tile_pool(name="big", bufs=1) as bigp,
            tc.tile_pool(name="ebuf", bufs=3) as ebufp,
            tc.tile_pool(name="work", bufs=6) as workp,
            tc.tile_pool(name="zs", bufs=6) as zp,
            tc.tile_pool(name="pp", bufs=2, space="PSUM") as pp,
            tc.tile_pool(name="sp", bufs=2, space="PSUM") as sp,
            tc.tile_pool(name="up", bufs=2, space="PSUM") as up,
        ):
            # ---- load W slices, x^T (c on partitions), diagonal masks.
            # Issue order: first matmuls need wk/wq + xt token-chunk 0;
            # the rest of xt streams in chunk by chunk behind them. ----
            wq = constp.tile([P, CS, EG], bf16, name="wq")
            wk = constp.tile([P, CS, EG], bf16, name="wk")
            wv = constp.tile([P, CS, EG], bf16, name="wv")
            wo = constp.tile([P, 2, D], bf16, name="wo")
            md = constp.tile([P, 2, 2, P], bf16, name="md")
            wk_r = wk_d[:].rearrange("p (s e) -> p s e", s=CS)
            nc.sync.dma_start(wk[:, 0:2, :], wk_r[:, 0:2, :])
            nc.sync.dma_start(wk[:, 2:CS, :], wk_r[:, 2:CS, :])

            xt = bigp.tile([P, CS, T], bf16, name="xt")
            xt_r = xt_d[:].rearrange("(s p) t -> p s t", p=P)
            # token-chunk 0 lands slice-by-slice right behind wk so the
            # first projection matmuls start ~2us in; wq/md/wv follow in
            # first-use order
            for s in range(CS):
                nc.sync.dma_start(xt[:, s, 0:TCH], xt_r[:, s, 0:TCH])
            nc.sync.dma_start(wq, wq_d[:].rearrange("p (s e) -> p s e", s=CS))
            nc.sync.dma_start(md, md_d[:])
            nc.sync.dma_start(wv, wv_d[:].rearrange("p (s e) -> p s e", s=CS))
            for tcc in range(1, NTC):
                nc.sync.dma_start(xt[:, :, tcc * TCH:(tcc + 1) * TCH],
                                  xt_r[:, :, tcc * TCH:(tcc + 1) * TCH])
            nc.sync.dma_start(wo, wo_d[:].rearrange("p (s e) -> p s e", s=2))

            env = dict(xt=xt, wq=wq, wk=wk, wv=wv, wo=wo, md=md, y_d=y_d,
                       bigp=bigp, ebufp=ebufp, workp=workp, zp=zp,
                       pp=pp, sp=sp, up=up)
            if loop_reps:
                with tc.For_i(0, loop_reps, 1):
                    _emit_body(nc, env, variant)
            else:
                _emit_body(nc, env, variant)

    nc.compile()
    return nc


def get_nc(loop_reps=0, variant="full"):
    key = (loop_reps, variant)
    if key not in _nc_cache:
        _nc_cache[key] = _build_nc(loop_reps, variant)
    return _nc_cache[key]


def _diag_masks():
    """[P, block, hh, 128] triangles: block 0 = key block 2qc against the
    chunk's first 128 queries; block 1 = key block 2qc+1 against the
    chunk's upper 128 queries. (Everything else on the diagonal pair is
    either fully valid or never computed.) Same mask for both heads."""
    jk = np.arange(P)[:, None]
    i = np.arange(P)[None, :]
    md = np.empty((P, 2, 2, P), np.float32)
    md[:, 0, :, :] = (jk <= i)[:, None]
    md[:, 1, :, :] = (jk <= i)[:, None]
    return md


def _pack_pm(w, nsub):
    """[nsub*128, F] -> partition-major [128, nsub*F] (row p = concat over
    sub-blocks of row sub*128+p) so the device DMA is contiguous per
    partition."""
    n, f = w.shape
    assert n == nsub * P
    return np.ascontiguousarray(
        w.reshape(nsub, P, f).transpose(1, 0, 2).reshape(P, nsub * f))


def prep_inputs(x, Wq, Wk, Wv, Wo):
    import ml_dtypes
    bf16 = ml_dtypes.bfloat16
    x = np.asarray(x, np.float32)
    md = _diag_masks()
    wqt = np.asarray(Wq, np.float32).T    # [c, e]
    wkt = np.asarray(Wk, np.float32).T
    wvt = np.asarray(Wv, np.float32).T
    wot = np.asarray(Wo, np.float32).T    # [c, e]

    in_maps = []
    for c in range(NCORES):
        b, g = divmod(c, NG)
        e0 = g * EG
        in_maps.append({
            "xt": np.ascontiguousarray(x[b].T).astype(bf16),
            "wqt": _pack_pm(wqt[:, e0:e0 + EG], CS).astype(bf16),
            "wkt": _pack_pm(wkt[:, e0:e0 + EG], CS).astype(bf16),
            "wvt": _pack_pm(wvt[:, e0:e0 + EG], CS).astype(bf16),
            "wot": _pack_pm(wot[e0:e0 + EG, :], 2).astype(bf16),
            "maskd": md.astype(bf16),
        })
    return in_maps


def gather_output(results):
    ys = [np.asarray(r["y"], np.float32) for r in results]
    out = np.empty((B, T, D), np.float32)
    for b in range(B):
        out[b] = sum(ys[b * NG:(b + 1) * NG])
    return out


last_run = {}


def kernel(x, Wq, Wk, Wv, Wo, _trace=False):
    from concourse.bass_utils import run_bass_kernel_spmd

    nc = get_nc()
    in_maps = prep_inputs(x, Wq, Wk, Wv, Wo)
    res = run_bass_kernel_spmd(
        nc, in_maps, core_ids=list(range(NCORES)), trace=_trace)
    last_run["results"] = res
    return gather_output(res.results)
